# revision 18
# baseline (speedup 1.0000x reference)
"""DSNT + leaky-integrator kernel for Trainium2 (8 NeuronCores, SPMD).

Math (matches the reference):
  px[w] = linspace(-1, 1, W);  py[h] = linspace(-1, 1, H)
  co_1[t] = sum_{h,w} x[t,h,w] * px[w]      (expected x-coordinate)
  co_2[t] = sum_{h,w} x[t,h,w] * py[h]      (expected y-coordinate)
  cos[t]  = (co_2[t], co_1[t])
  LI scan over t:  s = s - s*li_tm + cos[t]  ->  out[t] = s

Strategy:
  - Shard T=512 across 8 cores (64 frames each, ~75 MB per core); the
    DSNT reduction is embarrassingly parallel and memory-bound.
  - Per core, frames are processed in groups of 4 (1920 rows = 128
    partitions x 15 rows), so every DMA uses all 128 partitions with one
    contiguous 38.4 KB descriptor per partition -- measured ~2x DMA
    bandwidth vs any <128-partition layout on this part.
  - TensorE contracts the partition dim: for each of the 15 row-slots j,
    a [128, 8] stationary matrix (per frame g: col 2g = py at that row,
    col 2g+1 = 1, zero outside frame g's partition range) against moving
    [128, 320] x slices, accumulating PSUM [8, 320] x2 over j.
    Row 2g = sum_h py*x, row 2g+1 = sum_h x  (per w, frame g).
  - VectorE multiplies PSUM by [ones; px] rows and reduces along w ->
    (co_2, co_1) pairs, collected in an [8, 16] tile, DMA'd out once.
  - The 2-element LI recurrence is O(T) scalar work, done on host on the
    gathered [512, 2] cos values.
"""

import numpy as np
from contextlib import ExitStack

import concourse.bass as bass
import concourse.bacc as bacc
import concourse.tile as tile
from concourse import mybir
from concourse.bass_utils import run_bass_kernel_spmd

N_CORES = 8
T, H, W = 512, 480, 640
TL = T // N_CORES            # 64 frames per core
FG = 4                       # frames per group
NG = TL // FG                # 16 groups per core
RP = FG * H // 128           # 15 rows per partition
PF = 128 // FG               # 32 partitions per frame
WH = W // 2                  # 320, one PSUM bank per half
F32 = mybir.dt.float32

# Config knobs (test harness may override before first kernel() call).
USE_F32R = True              # f32r matmuls: 4x PE throughput, fp32-grade accuracy (measured)
X_BUFS = 4
TRACE = False
TRACE_CORES = None
LAST_RESULT = None

_NC_CACHE = {}


def _build_nc(use_f32r):
    nc = bacc.Bacc(
        "TRN2", target_bir_lowering=False, debug=False, num_devices=N_CORES
    )
    mmdt = mybir.dt.float32r if use_f32r else F32
    x = nc.dram_tensor("x", [TL * H * W], mmdt, kind="ExternalInput")
    res = nc.dram_tensor("res", [2 * FG, NG], F32, kind="ExternalOutput")

    px = np.linspace(-1.0, 1.0, W).astype(np.float32)           # [W]
    py = np.linspace(-1.0, 1.0, H).astype(np.float32)           # [H]
    # Stationary weights: [p, j, m] with m = 2*FG columns.
    wts_np = np.zeros((128, RP, 2 * FG), dtype=np.float32)
    for p in range(128):
        g, a = divmod(p, PF)
        for j in range(RP):
            wts_np[p, j, 2 * g] = py[a * RP + j]
            wts_np[p, j, 2 * g + 1] = 1.0
    # PSUM evacuation coefficients: row 2g -> ones (co_2), 2g+1 -> px (co_1)
    coeff_np = np.tile(
        np.stack([np.ones(W, np.float32), px], axis=0), (FG, 1)
    )                                                            # [8, W]
    wts_d = nc.inline_tensor(np.ascontiguousarray(wts_np), name="wts_const")
    coeff_d = nc.inline_tensor(np.ascontiguousarray(coeff_np), name="coeff_const")

    GE = FG * H * W                                              # elems per group

    with tile.TileContext(nc) as tc, ExitStack() as ctx:
        consts = ctx.enter_context(tc.tile_pool(name="consts", bufs=1))
        xpool = ctx.enter_context(tc.tile_pool(name="xtiles", bufs=X_BUFS))
        scr = ctx.enter_context(tc.tile_pool(name="scratch", bufs=3))
        psum = ctx.enter_context(tc.tile_pool(name="psum", bufs=4, space="PSUM"))
        outp = ctx.enter_context(tc.tile_pool(name="outp", bufs=1))

        lhsT_f32 = consts.tile([128, RP, 2 * FG], F32, name="lhsT_f32")
        nc.gpsimd.dma_start(out=lhsT_f32, in_=wts_d[:])
        if use_f32r:
            lhsT = consts.tile([128, RP, 2 * FG], mmdt, name="lhsT")
            nc.vector.tensor_copy(lhsT, lhsT_f32)
        else:
            lhsT = lhsT_f32
        coeff = consts.tile([2 * FG, W], F32, name="coeff")
        nc.gpsimd.dma_start(out=coeff, in_=coeff_d[:])

        resbuf = outp.tile([2 * FG, NG], F32, name="resbuf")

        for g in range(NG):
            xt = xpool.tile([128, RP, W], mmdt, tag="xt", name="xt")
            nc.sync.dma_start(
                out=xt,
                in_=x[g * GE : (g + 1) * GE].rearrange("(p e) -> p e", p=128),
            )
            pf = [psum.tile([2 * FG, WH], F32, tag=f"pf{h}", name=f"pf{h}")
                  for h in range(2)]
            for hw in range(2):
                sl = slice(hw * WH, (hw + 1) * WH)
                for j in range(RP):
                    nc.tensor.matmul(
                        pf[hw],
                        lhsT[:, j, :],
                        xt[:, j, sl],
                        start=(j == 0),
                        stop=(j == RP - 1),
                    )
            tmp = scr.tile([2 * FG, W], F32, tag="tmp", name="tmp")
            for hw in range(2):
                sl = slice(hw * WH, (hw + 1) * WH)
                nc.vector.tensor_mul(tmp[:, sl], pf[hw], coeff[:, sl])
            nc.vector.tensor_reduce(
                out=resbuf[:, g : g + 1],
                in_=tmp,
                axis=mybir.AxisListType.X,
                op=mybir.AluOpType.add,
            )

        nc.sync.dma_start(out=res[:], in_=resbuf)

    nc.finalize()
    return nc


def _get_nc():
    key = ("f32r" if USE_F32R else "f32", X_BUFS)
    if key not in _NC_CACHE:
        _NC_CACHE[key] = _build_nc(USE_F32R)
    return _NC_CACHE[key]


def kernel(x, li_tm, state):
    global LAST_RESULT
    x = np.ascontiguousarray(np.asarray(x, dtype=np.float32))
    li_tm = np.asarray(li_tm, dtype=np.float32)
    state = np.asarray(state, dtype=np.float32)
    assert x.shape == (T, H, W)

    nc = _get_nc()
    xf = x.reshape(N_CORES, TL * H * W)
    in_maps = [{"x": xf[i]} for i in range(N_CORES)]
    kwargs = {}
    if TRACE:
        kwargs["trace"] = True
        if TRACE_CORES is not None:
            kwargs["trace_cores"] = list(TRACE_CORES)
    r = run_bass_kernel_spmd(nc, in_maps, list(range(N_CORES)), **kwargs)
    LAST_RESULT = r
    # res[i] is [8, NG]: rows (2g, 2g+1) = (co_2, co_1) of frame gidx*FG+g
    cos = np.concatenate(
        [
            r.results[i]["res"].reshape(FG, 2, NG).transpose(2, 0, 1).reshape(TL, 2)
            for i in range(N_CORES)
        ],
        axis=0,
    ).astype(np.float32)

    # Leaky integrator (tiny 2-element recurrence), fp32 like the reference.
    s = state.copy()
    out = np.empty((T, 2), dtype=np.float32)
    for t in range(T):
        s = s - s * li_tm + cos[t]
        out[t] = s
    return out, out[-1].copy()


# revision 19
# speedup vs baseline: 2.1331x; 2.1331x over previous
"""DSNT + leaky-integrator kernel for Trainium2 (8 NeuronCores, SPMD).

Math (matches the reference):
  px[w] = linspace(-1, 1, W);  py[h] = linspace(-1, 1, H)
  co_1[t] = sum_{h,w} x[t,h,w] * px[w]      (expected x-coordinate)
  co_2[t] = sum_{h,w} x[t,h,w] * py[h]      (expected y-coordinate)
  cos[t]  = (co_2[t], co_1[t])
  LI scan over t:  s = s - s*li_tm + cos[t]  ->  out[t] = s

Strategy:
  - Shard T=512 across 8 cores (64 frames each, ~75 MB per core); the
    DSNT reduction is embarrassingly parallel and memory-bound.
  - Per core, frames are processed in groups of 4 (1920 rows = 128
    partitions x 15 rows), so every DMA uses all 128 partitions with one
    contiguous run per partition -- measured ~2x DMA bandwidth vs any
    <128-partition layout on this part.
  - TensorE contracts the partition dim: for each of the 15 row-slots j,
    a [128, m] stationary matrix (per frame g: a py column, a ones
    column, zero outside frame g's partition range) against moving
    [128, 320] x slices, accumulating PSUM [m, 320] x2 over j.
  - VectorE multiplies PSUM rows by [1 | px] coefficients and reduces
    along w -> per-frame scalars collected in a small tile, DMA'd out.
  - Variants: "f32" (exact, PE-bound ~290us), "f32r" (full-rate fp32r
    matmuls, fp32-grade accuracy, ~209us, DMA-bound), "f16" (x cast to
    fp16 on host, halves DMA bytes; py split into fp16 hi/lo columns so
    weight precision stays ~2^-22, only x quantization adds
    ~1.5e-4-of-scale error).
  - The 2-element LI recurrence is O(T) scalar work, done on host on the
    gathered [512, 2] cos values.
"""

import numpy as np
from contextlib import ExitStack

import concourse.bass as bass
import concourse.bacc as bacc
import concourse.tile as tile
from concourse import mybir
from concourse.bass_utils import run_bass_kernel_spmd

N_CORES = 8
T, H, W = 512, 480, 640
TL = T // N_CORES            # 64 frames per core
FG = 4                       # frames per group
NG = TL // FG                # 16 groups per core
RP = FG * H // 128           # 15 rows per partition
PF = 128 // FG               # 32 partitions per frame
WH = W // 2                  # 320, one PSUM bank per half
F32 = mybir.dt.float32
LO_SCALE = 2048.0            # fp16 py_lo column scale

# Config knobs (test harness may override before first kernel() call).
VARIANT = "f32r"             # "f32" | "f32r" | "f16"
X_BUFS = 3
TRACE = False
TRACE_CORES = None
LAST_RESULT = None

_NC_CACHE = {}


def _build_nc(variant, x_bufs):
    nc = bacc.Bacc(
        "TRN2", target_bir_lowering=False, debug=False, num_devices=N_CORES
    )
    mmdt = {
        "f32": F32,
        "f32r": mybir.dt.float32r,
        "f16": mybir.dt.float16,
    }[variant]
    x = nc.dram_tensor("x", [TL * H * W], mmdt, kind="ExternalInput")

    px = np.linspace(-1.0, 1.0, W).astype(np.float32)           # [W]
    py = np.linspace(-1.0, 1.0, H).astype(np.float32)           # [H]

    # Stationary weights [p, j, m] and PSUM coefficients [m, W].
    if variant == "f16":
        py_hi = py.astype(np.float16)
        py_lo = ((py.astype(np.float64) - py_hi.astype(np.float64)) * LO_SCALE)
        py_cols = [py_hi.astype(np.float32), py_lo.astype(np.float32)]
        wdt = np.float16
    else:
        py_cols = [py]
        wdt = np.float32
    npy = len(py_cols)
    M = (npy + 1) * FG
    wts_np = np.zeros((128, RP, M), dtype=np.float32)
    coeff_np = np.zeros((M, W), dtype=np.float32)
    for p in range(128):
        g, a = divmod(p, PF)
        for j in range(RP):
            h = a * RP + j
            for k, col in enumerate(py_cols):
                wts_np[p, j, (npy + 1) * g + k] = col[h]
            wts_np[p, j, (npy + 1) * g + npy] = 1.0
    for g in range(FG):
        for k in range(npy):
            coeff_np[(npy + 1) * g + k, :] = 1.0       # sum the py rows as-is
        coeff_np[(npy + 1) * g + npy, :] = px          # colsum row -> co_1
    wts_d = nc.inline_tensor(
        np.ascontiguousarray(wts_np.astype(wdt)), name="wts_const"
    )
    coeff_d = nc.inline_tensor(np.ascontiguousarray(coeff_np), name="coeff_const")

    GE = FG * H * W                                              # elems per group

    with tile.TileContext(nc) as tc, ExitStack() as ctx:
        consts = ctx.enter_context(tc.tile_pool(name="consts", bufs=1))
        xpool = ctx.enter_context(tc.tile_pool(name="xtiles", bufs=x_bufs))
        scr = ctx.enter_context(tc.tile_pool(name="scratch", bufs=3))
        psum = ctx.enter_context(tc.tile_pool(name="psum", bufs=4, space="PSUM"))
        outp = ctx.enter_context(tc.tile_pool(name="outp", bufs=1))

        wtile_dt = mybir.dt.float16 if variant == "f16" else F32
        lhsT_raw = consts.tile([128, RP, M], wtile_dt, name="lhsT_raw")
        nc.gpsimd.dma_start(out=lhsT_raw, in_=wts_d[:])
        if variant == "f32r":
            lhsT = consts.tile([128, RP, M], mmdt, name="lhsT")
            nc.vector.tensor_copy(lhsT, lhsT_raw)
        else:
            lhsT = lhsT_raw
        coeff = consts.tile([M, W], F32, name="coeff")
        nc.gpsimd.dma_start(out=coeff, in_=coeff_d[:])

        resbuf = outp.tile([M, NG], F32, name="resbuf")

        for g in range(NG):
            xt = xpool.tile([128, RP, W], mmdt, tag="xt", name="xt")
            nc.sync.dma_start(
                out=xt,
                in_=x[g * GE : (g + 1) * GE].rearrange("(p e) -> p e", p=128),
            )
            pf = [psum.tile([M, WH], F32, tag=f"pf{h}", name=f"pf{h}")
                  for h in range(2)]
            for hw in range(2):
                sl = slice(hw * WH, (hw + 1) * WH)
                for j in range(RP):
                    nc.tensor.matmul(
                        pf[hw],
                        lhsT[:, j, :],
                        xt[:, j, sl],
                        start=(j == 0),
                        stop=(j == RP - 1),
                    )
            tmp = scr.tile([M, W], F32, tag="tmp", name="tmp")
            for hw in range(2):
                sl = slice(hw * WH, (hw + 1) * WH)
                nc.vector.tensor_mul(tmp[:, sl], pf[hw], coeff[:, sl])
            nc.vector.tensor_reduce(
                out=resbuf[:, g : g + 1],
                in_=tmp,
                axis=mybir.AxisListType.X,
                op=mybir.AluOpType.add,
            )

        res = nc.dram_tensor("res", [M, NG], F32, kind="ExternalOutput")
        nc.sync.dma_start(out=res[:], in_=resbuf)

    nc.finalize()
    return nc


def _get_nc():
    key = (VARIANT, X_BUFS)
    if key not in _NC_CACHE:
        _NC_CACHE[key] = _build_nc(VARIANT, X_BUFS)
    return _NC_CACHE[key]


def kernel(x, li_tm, state):
    global LAST_RESULT
    x = np.ascontiguousarray(np.asarray(x, dtype=np.float32))
    li_tm = np.asarray(li_tm, dtype=np.float32)
    state = np.asarray(state, dtype=np.float32)
    assert x.shape == (T, H, W)

    nc = _get_nc()
    if VARIANT == "f16":
        xf = x.astype(np.float16).reshape(N_CORES, TL * H * W)
    else:
        xf = x.reshape(N_CORES, TL * H * W)
    in_maps = [{"x": xf[i]} for i in range(N_CORES)]
    kwargs = {}
    if TRACE:
        kwargs["trace"] = True
        if TRACE_CORES is not None:
            kwargs["trace_cores"] = list(TRACE_CORES)
    r = run_bass_kernel_spmd(nc, in_maps, list(range(N_CORES)), **kwargs)
    LAST_RESULT = r

    # res[i] is [M, NG]; per frame g the row block gives co_2 (npy rows to
    # sum, the lo row downscaled) and co_1.
    npy = 2 if VARIANT == "f16" else 1
    stride = npy + 1
    cos = np.empty((T, 2), dtype=np.float64)
    for i in range(N_CORES):
        rr = r.results[i]["res"].astype(np.float64)   # [M, NG]
        for g in range(FG):
            co2 = rr[stride * g]
            if npy == 2:
                co2 = co2 + rr[stride * g + 1] / LO_SCALE
            co1 = rr[stride * g + npy]
            idx = i * TL + np.arange(NG) * FG + g
            cos[idx, 0] = co2
            cos[idx, 1] = co1
    cos = cos.astype(np.float32)

    # Leaky integrator (tiny 2-element recurrence), fp32 like the reference.
    s = state.copy()
    out = np.empty((T, 2), dtype=np.float32)
    for t in range(T):
        s = s - s * li_tm + cos[t]
        out[t] = s
    return out, out[-1].copy()


# revision 26
# speedup vs baseline: 2.1536x; 1.0096x over previous
"""DSNT + leaky-integrator kernel for Trainium2 (8 NeuronCores, SPMD).

Math (matches the reference):
  px[w] = linspace(-1, 1, W);  py[h] = linspace(-1, 1, H)
  co_1[t] = sum_{h,w} x[t,h,w] * px[w]      (expected x-coordinate)
  co_2[t] = sum_{h,w} x[t,h,w] * py[h]      (expected y-coordinate)
  cos[t]  = (co_2[t], co_1[t])
  LI scan over t:  s = s - s*li_tm + cos[t]  ->  out[t] = s

Strategy:
  - Shard T=512 across 8 cores (64 frames each, ~75 MB per core); the
    DSNT reduction is embarrassingly parallel and memory-bound.
  - Per core, frames are processed in groups of 4 (1920 rows = 128
    partitions x 15 rows), so every DMA uses all 128 partitions with one
    contiguous run per partition -- measured ~2x DMA bandwidth vs any
    <128-partition layout on this part.
  - TensorE contracts the partition dim: for each of the 15 row-slots j,
    a [128, m] stationary matrix (per frame g: a py column, a ones
    column, zero outside frame g's partition range) against moving
    [128, 320] x slices, accumulating PSUM [m, 320] x2 over j.
  - VectorE multiplies PSUM rows by [1 | px] coefficients and reduces
    along w -> per-frame scalars collected in a small tile, DMA'd out.
  - Variants: "f32" (exact, PE-bound ~290us), "f32r" (full-rate fp32r
    matmuls, fp32-grade accuracy, ~209us, DMA-bound), "f16" (x cast to
    fp16 on host, halves DMA bytes; py split into fp16 hi/lo columns so
    weight precision stays ~2^-22, only x quantization adds
    ~1.5e-4-of-scale error).
  - The 2-element LI recurrence is O(T) scalar work, done on host on the
    gathered [512, 2] cos values.
"""

import numpy as np
from contextlib import ExitStack

import concourse.bass as bass
import concourse.bacc as bacc
import concourse.tile as tile
from concourse import mybir
from concourse.bass_utils import run_bass_kernel_spmd

N_CORES = 8
T, H, W = 512, 480, 640
TL = T // N_CORES            # 64 frames per core
FG = 4                       # frames per group
NG = TL // FG                # 16 groups per core
RP = FG * H // 128           # 15 rows per partition
PF = 128 // FG               # 32 partitions per frame
WH = W // 2                  # 320, one PSUM bank per half
F32 = mybir.dt.float32
LO_SCALE = 2048.0            # fp16 py_lo column scale

# Config knobs (test harness may override before first kernel() call).
VARIANT = "f16"              # "f32" | "f32r" | "f16"
ORDER = "hw"                 # matmul loop order: "hw" (half-outer) | "j"
X_BUFS = 4
TRACE = False
TRACE_CORES = None
LAST_RESULT = None

_NC_CACHE = {}


def _build_nc(variant, x_bufs, order="hw"):
    nc = bacc.Bacc(
        "TRN2", target_bir_lowering=False, debug=False, num_devices=N_CORES
    )
    mmdt = {
        "f32": F32,
        "f32r": mybir.dt.float32r,
        "f16": mybir.dt.float16,
    }[variant]
    x = nc.dram_tensor("x", [TL * H * W], mmdt, kind="ExternalInput")

    px = np.linspace(-1.0, 1.0, W).astype(np.float32)           # [W]
    py = np.linspace(-1.0, 1.0, H).astype(np.float32)           # [H]

    # Stationary weights [p, j, m] and PSUM coefficients [m, W].
    if variant == "f16":
        py_hi = py.astype(np.float16)
        py_lo = ((py.astype(np.float64) - py_hi.astype(np.float64)) * LO_SCALE)
        py_cols = [py_hi.astype(np.float32), py_lo.astype(np.float32)]
        wdt = np.float16
    else:
        py_cols = [py]
        wdt = np.float32
    npy = len(py_cols)
    M = (npy + 1) * FG
    wts_np = np.zeros((128, RP, M), dtype=np.float32)
    coeff_np = np.zeros((M, W), dtype=np.float32)
    for p in range(128):
        g, a = divmod(p, PF)
        for j in range(RP):
            h = a * RP + j
            for k, col in enumerate(py_cols):
                wts_np[p, j, (npy + 1) * g + k] = col[h]
            wts_np[p, j, (npy + 1) * g + npy] = 1.0
    for g in range(FG):
        for k in range(npy):
            coeff_np[(npy + 1) * g + k, :] = 1.0       # sum the py rows as-is
        coeff_np[(npy + 1) * g + npy, :] = px          # colsum row -> co_1
    wts_d = nc.inline_tensor(
        np.ascontiguousarray(wts_np.astype(wdt)), name="wts_const"
    )
    coeff_d = nc.inline_tensor(np.ascontiguousarray(coeff_np), name="coeff_const")

    GE = FG * H * W                                              # elems per group

    with tile.TileContext(nc) as tc, ExitStack() as ctx:
        consts = ctx.enter_context(tc.tile_pool(name="consts", bufs=1))
        xpool = ctx.enter_context(tc.tile_pool(name="xtiles", bufs=x_bufs))
        scr = ctx.enter_context(tc.tile_pool(name="scratch", bufs=3))
        psum = ctx.enter_context(tc.tile_pool(name="psum", bufs=4, space="PSUM"))
        outp = ctx.enter_context(tc.tile_pool(name="outp", bufs=1))

        wtile_dt = mybir.dt.float16 if variant == "f16" else F32
        lhsT_raw = consts.tile([128, RP, M], wtile_dt, name="lhsT_raw")
        nc.gpsimd.dma_start(out=lhsT_raw, in_=wts_d[:])
        if variant == "f32r":
            lhsT = consts.tile([128, RP, M], mmdt, name="lhsT")
            nc.vector.tensor_copy(lhsT, lhsT_raw)
        else:
            lhsT = lhsT_raw
        coeff = consts.tile([M, W], F32, name="coeff")
        nc.gpsimd.dma_start(out=coeff, in_=coeff_d[:])

        resbuf = outp.tile([M, NG], F32, name="resbuf")

        for g in range(NG):
            xt = xpool.tile([128, RP, W], mmdt, tag="xt", name="xt")
            nc.sync.dma_start(
                out=xt,
                in_=x[g * GE : (g + 1) * GE].rearrange("(p e) -> p e", p=128),
            )
            pf = [psum.tile([M, WH], F32, tag=f"pf{h}", name=f"pf{h}")
                  for h in range(2)]
            mm_iter = (
                [(hw, j) for hw in range(2) for j in range(RP)]
                if order == "hw"
                else [(hw, j) for j in range(RP) for hw in range(2)]
            )
            for hw, j in mm_iter:
                sl = slice(hw * WH, (hw + 1) * WH)
                nc.tensor.matmul(
                    pf[hw],
                    lhsT[:, j, :],
                    xt[:, j, sl],
                    start=(j == 0),
                    stop=(j == RP - 1),
                )
            tmp = scr.tile([M, W], F32, tag="tmp", name="tmp")
            for hw in range(2):
                sl = slice(hw * WH, (hw + 1) * WH)
                nc.vector.tensor_mul(tmp[:, sl], pf[hw], coeff[:, sl])
            nc.vector.tensor_reduce(
                out=resbuf[:, g : g + 1],
                in_=tmp,
                axis=mybir.AxisListType.X,
                op=mybir.AluOpType.add,
            )

        res = nc.dram_tensor("res", [M, NG], F32, kind="ExternalOutput")
        nc.sync.dma_start(out=res[:], in_=resbuf)

    nc.finalize()
    return nc


def _get_nc():
    key = (VARIANT, X_BUFS, ORDER)
    if key not in _NC_CACHE:
        _NC_CACHE[key] = _build_nc(VARIANT, X_BUFS, ORDER)
    return _NC_CACHE[key]


def kernel(x, li_tm, state):
    global LAST_RESULT
    x = np.ascontiguousarray(np.asarray(x, dtype=np.float32))
    li_tm = np.asarray(li_tm, dtype=np.float32)
    state = np.asarray(state, dtype=np.float32)
    assert x.shape == (T, H, W)

    nc = _get_nc()
    if VARIANT == "f16":
        xf = x.astype(np.float16).reshape(N_CORES, TL * H * W)
    else:
        xf = x.reshape(N_CORES, TL * H * W)
    in_maps = [{"x": xf[i]} for i in range(N_CORES)]
    kwargs = {}
    if TRACE:
        kwargs["trace"] = True
        if TRACE_CORES is not None:
            kwargs["trace_cores"] = list(TRACE_CORES)
    r = None
    for attempt in range(2):
        try:
            r = run_bass_kernel_spmd(nc, in_maps, list(range(N_CORES)), **kwargs)
            break
        except Exception:
            if attempt == 1:
                raise
    LAST_RESULT = r

    # res[i] is [M, NG]; per frame g the row block gives co_2 (npy rows to
    # sum, the lo row downscaled) and co_1.
    npy = 2 if VARIANT == "f16" else 1
    stride = npy + 1
    cos = np.empty((T, 2), dtype=np.float64)
    for i in range(N_CORES):
        rr = r.results[i]["res"].astype(np.float64)   # [M, NG]
        for g in range(FG):
            co2 = rr[stride * g]
            if npy == 2:
                co2 = co2 + rr[stride * g + 1] / LO_SCALE
            co1 = rr[stride * g + npy]
            idx = i * TL + np.arange(NG) * FG + g
            cos[idx, 0] = co2
            cos[idx, 1] = co1
    cos = cos.astype(np.float32)

    # Leaky integrator (tiny 2-element recurrence), fp32 like the reference.
    s = state.copy()
    out = np.empty((T, 2), dtype=np.float32)
    for t in range(T):
        s = s - s * li_tm + cos[t]
        out[t] = s
    return out, out[-1].copy()
